# revision 8
# baseline (speedup 1.0000x reference)
"""Trainium2 Bass kernel for the DecoderAttentionModel problem.

Math (per batch b):
  cell0 = enc[b, -1, :]                                  [H]
  blend1[s, w] = sum_h enc[b, s, h] * W1[w, h]           [S, W]   (loop-invariant)
  recurrence over t (h0 = 0, carried state is the new cell state):
    gates = (b_ih + b_hh) + c_prev @ W_hh.T              [4H] (o-gate unused)
    c = sigmoid(f)*cell0 + sigmoid(i)*tanh(g)
    blend2[t, w] = c @ W2.T                              [W]
  score[t, s] = sum_w v[w] * tanh(blend1[s, w] + blend2[t, w])
  out[b, t, s] = log_softmax_s(score[t, s])

Sharding: data-parallel over batch, 8 batches per core on 8 cores.

Device pipeline per core (ACT-bound: B/8*T*S*W = 537M tanh at 128/cyc@1.2GHz):
  - encoder slice DMA'd transposed (bf16) -> encT [h, s]
  - blend1T [w, s] fp32 via PE matmuls (bf16 inputs)
  - tiny LSTM recurrence in transposed layout, blend2T computed per step
    into t-chunked tiles so attention can overlap the recurrence tail
  - per (b, t): ACT tanh(blend1T chunk + blend2T[:, t] as per-partition
    bias) -> bf16 [w, s]; PE matvec with the tanh tile as the stationary
    operand accumulating scoresT psum [s_local, (s_grp, t)]
  - per b: drain psum, PE-transpose to [t, s], softmax along free dim
    (exp with accumulate + ln + subtract; |score| <= 16 so no max needed),
    DMA out.

Everything is statically unrolled: no dynamic loops, no register-offset APs,
so the Tile scheduler overlaps recurrence / blend1 build / attention /
epilogue freely across engines.
"""
import sys
sys.path.insert(0, '/opt/trn_rl_repo')

import numpy as np
import ml_dtypes

import concourse.bass as bass
import concourse.bacc as bacc
import concourse.mybir as mybir
import concourse.tile as tile
from concourse import bass_utils

F32 = mybir.dt.float32
BF16 = mybir.dt.bfloat16
AF = mybir.ActivationFunctionType
BFNP = ml_dtypes.bfloat16

B, S, H, W, T = 64, 2048, 256, 256, 128
NCORES = 8
BPC = B // NCORES

TCHUNK = 16           # blend2 t-chunk tile size (enables recurrence/attention overlap)


def build_program(nrep=1):
    nc = bacc.Bacc("TRN2", target_bir_lowering=False, debug=False, num_devices=NCORES)
    enc_d = nc.dram_tensor("enc", (BPC, S, H), BF16, kind="ExternalInput")
    cell0_d = nc.dram_tensor("cell0", (128, 2, BPC), F32, kind="ExternalInput")
    whhT_d = nc.dram_tensor("whhT", (128, 2, 6, 128), BF16, kind="ExternalInput")
    brep_d = nc.dram_tensor("brep", (128, 6, BPC), F32, kind="ExternalInput")
    w1T_d = nc.dram_tensor("w1T", (128, 2, 2, 128), BF16, kind="ExternalInput")
    w2T_d = nc.dram_tensor("w2T", (128, 2, 2, 128), F32, kind="ExternalInput")
    vb_d = nc.dram_tensor("vb", (128, 2), BF16, kind="ExternalInput")
    ident_d = nc.dram_tensor("ident", (128, 128), F32, kind="ExternalInput")
    out_d = nc.dram_tensor("probs", (BPC, T, S), F32, kind="ExternalOutput")

    with tile.TileContext(nc) as tc:
        with tc.tile_pool(name="const", bufs=1) as cpool:
            cell0 = cpool.tile([128, 2, BPC], F32)
            nc.sync.dma_start(cell0[:], cell0_d.ap())
            whhT = cpool.tile([128, 2, 6, 128], BF16)
            nc.sync.dma_start(whhT[:], whhT_d.ap())
            brep = cpool.tile([128, 6, BPC], F32)
            nc.sync.dma_start(brep[:], brep_d.ap())
            w1T = cpool.tile([128, 2, 2, 128], BF16)
            nc.sync.dma_start(w1T[:], w1T_d.ap())
            w2T = cpool.tile([128, 2, 2, 128], F32)
            nc.sync.dma_start(w2T[:], w2T_d.ap())
            vb = cpool.tile([128, 2], BF16)
            nc.sync.dma_start(vb[:], vb_d.ap())
            ident = cpool.tile([128, 128], F32)
            nc.sync.dma_start(ident[:], ident_d.ap())

            # blend2T in t-chunked tiles: [w_p, w_chunk, b, t_local]
            nchunk = T // TCHUNK
            blend2 = [cpool.tile([128, 2, BPC, TCHUNK], F32, name=f"blend2_{g}")
                      for g in range(nchunk)]
            cstate = cpool.tile([128, 2, BPC], BF16)

            import contextlib
            rep_ctx = tc.For_i(0, nrep, 1) if nrep > 1 else contextlib.nullcontext()
            with rep_ctx:
                rep = 0
                # ---------------- LSTM recurrence ----------------
                with tc.tile_pool(name="rwork", bufs=2) as rpool, \
                     tc.tile_pool(name="rpsum", bufs=1, space="PSUM") as rps, \
                     tc.tile_pool(name="b2psum", bufs=1, space="PSUM") as b2ps:
                    nc.vector.memset(cstate[:], 0.0)
                    for i in range(T):
                        gps = rps.tile([128, 6, BPC], F32, tag="g", name=f"g{rep}_{i}")
                        for g in range(6):
                            for c in range(2):
                                nc.tensor.matmul(gps[:, g], whhT[:, c, g], cstate[:, c],
                                                 start=(c == 0), stop=(c == 1))
                        gb = rpool.tile([128, 6, BPC], F32, tag="gb", name=f"gb{rep}_{i}")
                        nc.vector.tensor_add(gb[:], gps[:], brep[:])
                        sgt = rpool.tile([128, 6, BPC], F32, tag="sgt", name=f"sgt{rep}_{i}")
                        nc.scalar.activation(sgt[:, 0:4], gb[:, 0:4], AF.Sigmoid)
                        nc.scalar.activation(sgt[:, 4:6], gb[:, 4:6], AF.Tanh)
                        tmp = rpool.tile([128, 2, BPC], F32, tag="tmp", name=f"tp{rep}_{i}")
                        nc.vector.tensor_mul(tmp[:], sgt[:, 0:2], sgt[:, 4:6])
                        cnew = rpool.tile([128, 2, BPC], F32, tag="cnew", name=f"cn{rep}_{i}")
                        nc.vector.tensor_mul(cnew[:], sgt[:, 2:4], cell0[:])
                        nc.vector.tensor_add(cnew[:], cnew[:], tmp[:])
                        nc.vector.tensor_copy(cstate[:], cnew[:])
                        # blend2 column for step i
                        bps = b2ps.tile([128, 2, BPC], F32, tag="b2", name=f"b2{rep}_{i}")
                        for wc in range(2):
                            for k in range(2):
                                nc.tensor.matmul(bps[:, wc], w2T[:, k, wc],
                                                 cnew[:, k], start=(k == 0), stop=(k == 1))
                        g_i, t_i = i // TCHUNK, i % TCHUNK
                        nc.vector.tensor_copy(blend2[g_i][:, :, :, t_i], bps[:])

                # ---------------- attention + softmax, per local batch ----------------
                with tc.tile_pool(name="encp", bufs=2) as epool, \
                     tc.tile_pool(name="b1p", bufs=2) as b1pool, \
                     tc.tile_pool(name="thp", bufs=3) as thpool, \
                     tc.tile_pool(name="scp", bufs=2) as scpool, \
                     tc.tile_pool(name="sTp", bufs=4) as sTpool, \
                     tc.tile_pool(name="escp", bufs=1) as escpool, \
                     tc.tile_pool(name="smp", bufs=2) as smpool, \
                     tc.tile_pool(name="pscore", bufs=4, space="PSUM") as pscore, \
                     tc.tile_pool(name="pwork", bufs=2, space="PSUM") as pwork:
                    for b in range(BPC):
                        encT = epool.tile([128, 2, S], BF16, tag="encT", name=f"encT{rep}_{b}")
                        for c in range(2):
                            nc.sync.dma_start_transpose(
                                encT[:, c, :], enc_d.ap()[b, :, 128 * c:128 * (c + 1)])
                        blend1 = b1pool.tile([128, 2, S], F32, tag="b1", name=f"b1{rep}_{b}")
                        for wc in range(2):
                            for n in range(4):
                                ps = pwork.tile([128, 512], F32, tag="pw",
                                                name=f"pw{rep}_{b}_{wc}_{n}")
                                for k in range(2):
                                    nc.tensor.matmul(ps[:], w1T[:, k, wc],
                                                     encT[:, k, 512 * n:512 * (n + 1)],
                                                     start=(k == 0), stop=(k == 1))
                                nc.vector.tensor_copy(blend1[:, wc, 512 * n:512 * (n + 1)],
                                                      ps[:])

                        scps = [pscore.tile([128, 512], F32, tag="scps",
                                            name=f"scps{rep}_{b}_{j}") for j in range(4)]
                        TB = 4       # t-steps per ACT instruction
                        for m in range(T // TB):
                            ths = []
                            for c in range(2):
                                th = thpool.tile([128, TB, S], BF16, tag=f"th{c}",
                                                 name=f"th{rep}_{b}_{m}_{c}")
                                for u in range(TB):
                                    i = TB * m + u
                                    g_i, t_i = i // TCHUNK, i % TCHUNK
                                    nc.vector.tensor_scalar(
                                        th[:, u, :], blend1[:, c, :],
                                        blend2[g_i][:, c, b, t_i:t_i + 1], None,
                                        mybir.AluOpType.add)
                                nc.scalar.activation(th[:], th[:], AF.Tanh)
                                ths.append(th)
                            for u in range(TB):
                                i = TB * m + u
                                for j in range(4):
                                    for q in range(4):
                                        sidx = 4 * j + q
                                        for c in range(2):
                                            col = 128 * q + i
                                            nc.tensor.matmul(
                                                scps[j][:, col:col + 1],
                                                ths[c][:, u, 128 * sidx:128 * (sidx + 1)],
                                                vb[:, c:c + 1],
                                                start=(c == 0), stop=(c == 1))

                        scores = scpool.tile([128, S], F32, tag="scores",
                                             name=f"sc{rep}_{b}")
                        for j in range(4):
                            sT = sTpool.tile([128, 512], F32, tag="sT",
                                             name=f"sT{rep}_{b}_{j}")
                            nc.vector.tensor_copy(sT[:], scps[j][:])
                            for q in range(4):
                                pt = pwork.tile([128, 128], F32, tag="pw",
                                                name=f"pt{rep}_{b}_{j}_{q}")
                                nc.tensor.transpose(pt[:], sT[:, 128 * q:128 * (q + 1)],
                                                    ident[:])
                                nc.vector.tensor_copy(
                                    scores[:, 128 * (4 * j + q):128 * (4 * j + q + 1)],
                                    pt[:])

                        esc = escpool.tile([128, S], F32, tag="esc", name=f"esc{rep}_{b}")
                        sums = smpool.tile([128, 1], F32, tag="sums", name=f"sm{rep}_{b}")
                        nc.scalar.activation(esc[:], scores[:], AF.Exp, accum_out=sums[:])
                        lse = smpool.tile([128, 1], F32, tag="lse", name=f"ls{rep}_{b}")
                        nc.scalar.activation(lse[:], sums[:], AF.Ln)
                        nc.vector.tensor_scalar(scores[:], scores[:], lse[:], None,
                                                mybir.AluOpType.subtract)
                        nc.sync.dma_start(out_d.ap()[b], scores[:])

    nc.compile()
    return nc


_prog = None


def _get_prog():
    global _prog
    if _prog is None:
        _prog = build_program()
    return _prog


def _prep_inputs(encoder_output, W_hh, b_ih, b_hh, W1, W2, vt):
    enc = np.asarray(encoder_output, dtype=np.float32)          # [B, S, H]
    W_hh = np.asarray(W_hh, dtype=np.float32)
    W1 = np.asarray(W1, dtype=np.float32)
    W2 = np.asarray(W2, dtype=np.float32)
    vt = np.asarray(vt, dtype=np.float32)
    bias = (np.asarray(b_ih, np.float32) + np.asarray(b_hh, np.float32))[:3 * H]

    enc_bf = enc.astype(BFNP)                                    # [B, S, H]
    # brep[p, g, b] = bias[g*128 + p]
    brep = np.ascontiguousarray(
        np.broadcast_to(bias.reshape(6, 128).T[:, :, None], (128, 6, BPC))
    ).astype(np.float32)
    # whhT[p, c, g, col] = W_hh[g*128+col, c*128+p]
    whhT = np.ascontiguousarray(
        W_hh[:3 * H].reshape(6, 128, 2, 128).transpose(3, 2, 0, 1)
    ).astype(BFNP)
    # w1T[p, k, m, col] = W1[m*128+col, k*128+p]
    w1T = np.ascontiguousarray(
        W1.reshape(2, 128, 2, 128).transpose(3, 2, 0, 1)
    ).astype(BFNP)
    w2T = np.ascontiguousarray(
        W2.reshape(2, 128, 2, 128).transpose(3, 2, 0, 1)
    ).astype(np.float32)
    vb = np.ascontiguousarray(vt[0].reshape(2, 128).T).astype(BFNP)
    ident = np.eye(128, dtype=np.float32)

    cell0 = enc[:, -1, :]                                        # [B, H] fp32
    in_maps = []
    for ci in range(NCORES):
        bsl = slice(ci * BPC, (ci + 1) * BPC)
        # cell0T[p, c, b] = cell0[b_global, c*128+p]
        c0 = np.ascontiguousarray(
            cell0[bsl].reshape(BPC, 2, 128).transpose(2, 1, 0)
        ).astype(np.float32)
        in_maps.append({
            "enc": np.ascontiguousarray(enc_bf[bsl]),
            "cell0": c0,
            "whhT": whhT,
            "brep": brep,
            "w1T": w1T,
            "w2T": w2T,
            "vb": vb,
            "ident": ident,
        })
    return in_maps


def run_on_device(in_maps):
    nc = _get_prog()
    return bass_utils.run_bass_kernel_spmd(nc, in_maps, core_ids=list(range(NCORES)))


def kernel(input, encoder_output, W_ih, W_hh, b_ih, b_hh, W1, W2, vt):
    # `input` and `W_ih` do not affect the output: the decoder input is all
    # zeros, so the input-side gate contribution reduces to the biases.
    in_maps = _prep_inputs(encoder_output, W_hh, b_ih, b_hh, W1, W2, vt)
    res = run_on_device(in_maps)
    out = np.concatenate([res.results[i]["probs"] for i in range(NCORES)], axis=0)
    return out
